# revision 8
# baseline (speedup 1.0000x reference)
"""Trainium2 Bass kernel: 8 independent 3x3 filters applied to every channel.

Reference op: x[B=8, C=32, 224, 224], W[1, 8, 3, 3], Bv[8]
  -> y[B, 8*C, 222, 222],  y[b, d*C+c, i, j] = sum_{u,v} x[b,c,i+u,j+v] W[0,d,u,v] + Bv[d]

Sharding: data-parallel over batch B across the 8 cores (core k takes x[k]).

Per-core formulation (all compute on TensorE):
  Output rows are processed in blocks of 32. For each block, filter-half dh
  (4 filters) and width-shift v in {0,1,2}:
    out[(d'*32+di), (img, j)] += sum_{r} LWq[r, d'*32+di] * TILE[r, img, j+v]
  where TILE is a full-height input tile (rows on partitions, base 0) and
  LWq[32q+di+u, d'*32+di] = W[0, 4*dh+d', u, v] is a banded weight matrix with
  the band positioned at the block's row offset inside the tile (q = p0/32).
  M is filter-major so each output channel owns a contiguous 32-partition
  slice of the accumulator, making per-channel output DMAs 3-dim APs.
  K = tile height (<=128), M = 128 = 32 out-rows x 4 filters, N = 444 = 2
  images x 222 (N>=256 keeps float32r matmuls at 1 cycle/row).
  The 3 v-shift matmuls accumulate in PSUM; bias is added during the
  PSUM->SBUF copy (DVE tensor_scalar); outputs collect in per-image SBUF
  tiles and leave in large batched DMAs.

Input tiles per 8-image group: rows 0:128, rows 96:224, rows 192:224.
Blocks (i0, tile, q): (0,T0,0) (32,T0,1) (64,T0,2) (96,T2,0) (128,T2,1)
(160,T2,2) + tail (192,T4,0) whose valid outputs are di<30 (rows 192..221).
"""

import os
import numpy as np

B, C, H, W_IN = 8, 32, 224, 224
ND, KS = 8, 3
HO, WO = 222, 222
NCORES = 8
GSZ = 8  # images per input-tile group

# (i0, tile_idx, q, K) ; tile bases: T0=0, T2=96, T4=192
BLOCKS = [
    (0, 0, 0, 128),
    (32, 0, 1, 128),
    (64, 0, 2, 128),
    (96, 1, 0, 128),
    (128, 1, 1, 128),
    (160, 1, 2, 128),
    (192, 2, 0, 32),  # tail: valid out rows di<30
]
NBLK = len(BLOCKS)
TILE_ROWS = [(0, 128), (96, 128), (192, 32)]

_PROG_CACHE = {}


def _build(mode: str, n_imgs: int):
    """Build+compile the per-core Bass program.

    mode: 'f32' (exact, 4 cyc/row), 'f32r' (relaxed fp32, 1 cyc/row @ N>=256),
          'bf16' (host-cast inputs).
    """
    import concourse.mybir as mybir
    import concourse.tile as tile
    from concourse import bacc

    dt = mybir.dt
    if mode == "bf16":
        io_dt = dt.bfloat16
    elif mode == "f32r":
        io_dt = dt.float32r
    else:
        io_dt = dt.float32

    n_groups = n_imgs // GSZ
    assert n_imgs % GSZ == 0

    nc = bacc.Bacc("TRN2", target_bir_lowering=False, debug=False)
    xin = nc.dram_tensor("xin", [n_imgs, H, W_IN], io_dt, kind="ExternalInput")
    lw = nc.dram_tensor("lw", [128, 3, 3, 2, 128], io_dt, kind="ExternalInput")
    bias = nc.dram_tensor("bias", [128, 2], dt.float32, kind="ExternalInput")
    yout = nc.dram_tensor("yout", [ND, n_imgs, HO, WO], dt.float32,
                          kind="ExternalOutput")

    with tile.TileContext(nc) as tc:
        with (
            tc.tile_pool(name="const", bufs=1) as constp,
            tc.tile_pool(name="inp", bufs=2) as inp,
            tc.tile_pool(name="outp", bufs=2) as outp,
            tc.tile_pool(name="psum", bufs=7, space="PSUM") as psp,
        ):
            lw_sb = constp.tile([128, 3, 3, 2, 128], io_dt)
            nc.sync.dma_start(lw_sb[:], lw[:])
            bias_sb = constp.tile([128, 2], dt.float32)
            nc.sync.dma_start(bias_sb[:], bias[:])

            for g in range(n_groups):
                g8 = g * GSZ
                tiles = []
                for ti, (r0, nr) in enumerate(TILE_ROWS):
                    t = inp.tile([nr, GSZ, W_IN], io_dt, name=f"t{ti}",
                                 tag=f"t{ti}")
                    nc.sync.dma_start(
                        t[:], xin[g8:g8 + GSZ, r0:r0 + nr, :].transpose([1, 0, 2]))
                    tiles.append(t)

                for pr in range(GSZ // 2):
                    # acc[dh][j]: per-image, per-filter-half output collector
                    acc = [
                        [outp.tile([128, NBLK * WO], dt.float32,
                                   name=f"acc{dh}{j}", tag=f"acc{dh}{j}")
                         for j in range(2)]
                        for dh in range(2)
                    ]
                    for bi, (i0, ti, q, kk) in enumerate(BLOCKS):
                        src = tiles[ti]
                        for dh in range(2):
                            ps = psp.tile([128, 2, WO], dt.float32, name="ps")
                            for v in range(3):
                                nc.tensor.matmul(
                                    ps[:],
                                    lw_sb[0:kk, q, v, dh, :],
                                    src[0:kk, 2 * pr:2 * pr + 2, v:v + WO],
                                    start=(v == 0),
                                    stop=(v == 2),
                                )
                            for j in range(2):
                                nc.vector.tensor_scalar_add(
                                    acc[dh][j][:, bi * WO:(bi + 1) * WO],
                                    ps[:, j, :],
                                    bias_sb[:, dh:dh + 1],
                                )
                    for j in range(2):
                        c = g8 + 2 * pr + j
                        for dh in range(2):
                            for dd in range(4):
                                ch = 4 * dh + dd
                                # rows 0..191 (blocks 0..5); SBUF walk
                                # (di, blk, j) <-> DRAM rows blk*32+di
                                main_ap = (
                                    yout[ch, c, 0:192, :]
                                    .rearrange("(blk di) j -> blk di j", di=32)
                                    .transpose([1, 0, 2])
                                )
                                nc.sync.dma_start(
                                    main_ap,
                                    acc[dh][j][32 * dd:32 * dd + 32, 0:6 * WO])
                                # rows 192..221: tail block, di<30 valid
                                nc.sync.dma_start(
                                    yout[ch, c, 192:222, :],
                                    acc[dh][j][32 * dd:32 * dd + 30, 6 * WO:])

    nc.compile()
    return nc


def _get_prog(mode: str, n_imgs: int = C):
    key = (mode, n_imgs)
    if key not in _PROG_CACHE:
        _PROG_CACHE[key] = _build(mode, n_imgs)
    return _PROG_CACHE[key]


def _host_weights(W: np.ndarray, Bv: np.ndarray, mode: str):
    """LW[32q+di+u, q, v, dh, d'*32+di] = W[0, 4dh+d', u, v];
    bias[d'*32+di, dh] = Bv[4dh+d']."""
    LW = np.zeros((128, 3, 3, 2, 128), np.float32)
    for q in range(3):
        for v in range(3):
            for dh in range(2):
                for dd in range(4):
                    for di in range(32):
                        for u in range(3):
                            r = 32 * q + di + u
                            if r < 128:
                                LW[r, q, v, dh, dd * 32 + di] = \
                                    W[0, 4 * dh + dd, u, v]
    bias = np.stack(
        [np.repeat(np.asarray(Bv[4 * dh:4 * dh + 4], np.float32), 32)
         for dh in range(2)], axis=1)
    if mode == "bf16":
        import ml_dtypes
        LW = LW.astype(ml_dtypes.bfloat16)
    return np.ascontiguousarray(LW), np.ascontiguousarray(bias)


def _cast_in(x: np.ndarray, mode: str):
    if mode == "bf16":
        import ml_dtypes
        return np.ascontiguousarray(x).astype(ml_dtypes.bfloat16)
    return np.ascontiguousarray(x, np.float32)


def kernel(x, W, Bv, mode: str | None = None, _trace: bool = False):
    from concourse.bass_utils import run_bass_kernel_spmd

    mode = mode or os.environ.get("DCONV_MODE", "f32r")
    x = np.asarray(x, np.float32)
    W = np.asarray(W, np.float32)
    Bv = np.asarray(Bv, np.float32)

    nc = _get_prog(mode)
    LW, bias = _host_weights(W, Bv, mode)
    in_maps = [
        {"xin": _cast_in(x[k], mode), "lw": LW, "bias": bias}
        for k in range(NCORES)
    ]
    res = run_bass_kernel_spmd(nc, in_maps, core_ids=list(range(NCORES)),
                               trace=_trace)
    y = np.stack(
        [np.asarray(res.results[k]["yout"]).reshape(ND * C, HO, WO)
         for k in range(NCORES)],
        axis=0,
    )
    if _trace:
        return y, res
    return y


# revision 11
# speedup vs baseline: 1.8517x; 1.8517x over previous
"""Trainium2 Bass kernel: 8 independent 3x3 filters applied to every channel.

Reference op: x[B=8, C=32, 224, 224], W[1, 8, 3, 3], Bv[8]
  -> y[B, 8*C, 222, 222],  y[b, d*C+c, i, j] = sum_{u,v} x[b,c,i+u,j+v] W[0,d,u,v] + Bv[d]

Sharding: data-parallel over batch B across the 8 cores (core k takes x[k]).

Per-core formulation (all compute on TensorE):
  Output rows are processed in blocks of 32. For each block, filter-half dh
  (4 filters) and width-shift v in {0,1,2}:
    out[(d'*32+di), (img, j)] += sum_{r} LWq[r, d'*32+di] * TILE[r, img, j+v]
  where TILE is a full-height input tile (rows on partitions, base 0) and
  LWq[32q+di+u, d'*32+di] = W[0, 4*dh+d', u, v] is a banded weight matrix with
  the band positioned at the block's row offset inside the tile (q = p0/32).
  M is filter-major so each output channel owns a contiguous 32-partition
  slice of the accumulator, making per-channel output DMAs 3-dim APs.
  K = tile height (<=128), M = 128 = 32 out-rows x 4 filters, N = 444 = 2
  images x 222 (N>=256 keeps float32r matmuls at 1 cycle/row).
  The 3 v-shift matmuls accumulate in PSUM; bias is added during the
  PSUM->SBUF copy (DVE tensor_scalar); outputs collect in per-image SBUF
  tiles and leave in large batched DMAs.

Input tiles per 8-image group: rows 0:128, rows 96:224, rows 192:224.
Blocks (i0, tile, q): (0,T0,0) (32,T0,1) (64,T0,2) (96,T2,0) (128,T2,1)
(160,T2,2) + tail (192,T4,0) whose valid outputs are di<30 (rows 192..221).
"""

import os
import numpy as np

B, C, H, W_IN = 8, 32, 224, 224
ND, KS = 8, 3
HO, WO = 222, 222
NCORES = 8
GSZ = 8  # images per input-tile group

# (i0, tile_idx, q, K) ; tile bases: T0=0, T2=96, T4=192
BLOCKS = [
    (0, 0, 0, 128),
    (32, 0, 1, 128),
    (64, 0, 2, 128),
    (96, 1, 0, 128),
    (128, 1, 1, 128),
    (160, 1, 2, 128),
    (192, 2, 0, 32),  # tail: valid out rows di<30
]
NBLK = len(BLOCKS)
TILE_ROWS = [(0, 128), (96, 128), (192, 32)]

_PROG_CACHE = {}


def _build(mode: str, n_imgs: int):
    """Build+compile the per-core Bass program.

    mode: 'f32' (exact, 4 cyc/row), 'f32r' (relaxed fp32, 1 cyc/row @ N>=256),
          'bf16' (host-cast inputs).
    """
    import concourse.mybir as mybir
    import concourse.tile as tile
    from concourse import bacc

    dt = mybir.dt
    if mode == "bf16":
        io_dt = dt.bfloat16
    elif mode == "f32r":
        io_dt = dt.float32r
    else:
        io_dt = dt.float32

    n_groups = n_imgs // GSZ
    assert n_imgs % GSZ == 0

    nc = bacc.Bacc("TRN2", target_bir_lowering=False, debug=False)
    xin = nc.dram_tensor("xin", [n_imgs, H, W_IN], io_dt, kind="ExternalInput")
    lw = nc.dram_tensor("lw", [128, 3, 3, 2, 128], io_dt, kind="ExternalInput")
    bias = nc.dram_tensor("bias", [128, 2], dt.float32, kind="ExternalInput")
    # rows padded 222->224: the tail block's di=30,31 garbage lands in the
    # pad rows, so each channel leaves in ONE contiguous [32, 7*222] DMA.
    yout = nc.dram_tensor("yout", [ND, n_imgs, NBLK * 32, WO], dt.float32,
                          kind="ExternalOutput")

    with tile.TileContext(nc) as tc:
        with (
            tc.tile_pool(name="const", bufs=1) as constp,
            tc.tile_pool(name="inp", bufs=2) as inp,
            tc.tile_pool(name="outp", bufs=2) as outp,
            tc.tile_pool(name="psum", bufs=7, space="PSUM") as psp,
        ):
            lw_sb = constp.tile([128, 3, 3, 2, 128], io_dt)
            nc.sync.dma_start(lw_sb[:], lw[:])
            bias_sb = constp.tile([128, 2], dt.float32)
            nc.sync.dma_start(bias_sb[:], bias[:])

            for g in range(n_groups):
                g8 = g * GSZ
                tiles = []
                for ti, (r0, nr) in enumerate(TILE_ROWS):
                    t = inp.tile([nr, GSZ, W_IN], io_dt, name=f"t{ti}",
                                 tag=f"t{ti}")
                    nc.sync.dma_start(
                        t[:], xin[g8:g8 + GSZ, r0:r0 + nr, :].transpose([1, 0, 2]))
                    tiles.append(t)

                for pr in range(GSZ // 2):
                    # acc[dh][j]: per-image, per-filter-half output collector
                    acc = [
                        [outp.tile([128, NBLK * WO], dt.float32,
                                   name=f"acc{dh}{j}", tag=f"acc{dh}{j}")
                         for j in range(2)]
                        for dh in range(2)
                    ]
                    for bi, (i0, ti, q, kk) in enumerate(BLOCKS):
                        src = tiles[ti]
                        for dh in range(2):
                            ps = psp.tile([128, 2, WO], dt.float32, name="ps")
                            for v in range(3):
                                nc.tensor.matmul(
                                    ps[:],
                                    lw_sb[0:kk, q, v, dh, :],
                                    src[0:kk, 2 * pr:2 * pr + 2, v:v + WO],
                                    start=(v == 0),
                                    stop=(v == 2),
                                )
                            for j in range(2):
                                nc.vector.tensor_scalar_add(
                                    acc[dh][j][:, bi * WO:(bi + 1) * WO],
                                    ps[:, j, :],
                                    bias_sb[:, dh:dh + 1],
                                )
                    for j in range(2):
                        c = g8 + 2 * pr + j
                        for dh in range(2):
                            for dd in range(4):
                                ch = 4 * dh + dd
                                # SBUF walk (di, blk, j) <-> DRAM rows
                                # blk*32+di of the padded [224, 222] channel
                                ch_ap = (
                                    yout[ch, c, :, :]
                                    .rearrange("(blk di) j -> blk di j", di=32)
                                    .transpose([1, 0, 2])
                                )
                                eng = nc.sync if (dd % 2 == 0) else nc.scalar
                                eng.dma_start(
                                    ch_ap,
                                    acc[dh][j][32 * dd:32 * dd + 32, :])

    nc.compile()
    return nc


def _get_prog(mode: str, n_imgs: int = C):
    key = (mode, n_imgs)
    if key not in _PROG_CACHE:
        _PROG_CACHE[key] = _build(mode, n_imgs)
    return _PROG_CACHE[key]


def _host_weights(W: np.ndarray, Bv: np.ndarray, mode: str):
    """LW[32q+di+u, q, v, dh, d'*32+di] = W[0, 4dh+d', u, v];
    bias[d'*32+di, dh] = Bv[4dh+d']."""
    LW = np.zeros((128, 3, 3, 2, 128), np.float32)
    for q in range(3):
        for v in range(3):
            for dh in range(2):
                for dd in range(4):
                    for di in range(32):
                        for u in range(3):
                            r = 32 * q + di + u
                            if r < 128:
                                LW[r, q, v, dh, dd * 32 + di] = \
                                    W[0, 4 * dh + dd, u, v]
    bias = np.stack(
        [np.repeat(np.asarray(Bv[4 * dh:4 * dh + 4], np.float32), 32)
         for dh in range(2)], axis=1)
    if mode == "bf16":
        import ml_dtypes
        LW = LW.astype(ml_dtypes.bfloat16)
    return np.ascontiguousarray(LW), np.ascontiguousarray(bias)


def _cast_in(x: np.ndarray, mode: str):
    if mode == "bf16":
        import ml_dtypes
        return np.ascontiguousarray(x).astype(ml_dtypes.bfloat16)
    return np.ascontiguousarray(x, np.float32)


def kernel(x, W, Bv, mode: str | None = None, _trace: bool = False):
    from concourse.bass_utils import run_bass_kernel_spmd

    mode = mode or os.environ.get("DCONV_MODE", "f32r")
    x = np.asarray(x, np.float32)
    W = np.asarray(W, np.float32)
    Bv = np.asarray(Bv, np.float32)

    nc = _get_prog(mode)
    LW, bias = _host_weights(W, Bv, mode)
    in_maps = [
        {"xin": _cast_in(x[k], mode), "lw": LW, "bias": bias}
        for k in range(NCORES)
    ]
    res = run_bass_kernel_spmd(nc, in_maps, core_ids=list(range(NCORES)),
                               trace=_trace)
    y = np.stack(
        [np.ascontiguousarray(
            np.asarray(res.results[k]["yout"])[:, :, :HO, :]
        ).reshape(ND * C, HO, WO) for k in range(NCORES)],
        axis=0,
    )
    if _trace:
        return y, res
    return y
